# revision 1
# baseline (speedup 1.0000x reference)
"""CrossAttentionFusion Trainium2 kernel.

Full inputs -> shard (batch x query-half) over 8 NeuronCores -> full output.

Per core (batch b = core//2, query half h = core%2, NH=2048 queries):
  Algebraic folding (host precompute):
    L[m,n] = K^T Q = x2^T (k_w^T q_w) x1 =: x2^T Q'   (K never materialized;
             terms constant in m cancel in softmax; x2^T k_w^T q_b folds
             into Q' channel bias)
    F_att   = v_w (x2 A_norm) + v_b  ->  M1 = (proj_w v_w) Z,  Z = x2 E
             (V never materialized; proj_w v_w and proj_w v_b precomputed)
  Device per 512-query block:
    L[m, n] = x2^T Q'                (fp32r matmuls, m on partitions)
    E = exp(L / 16)                  (ACT; no max subtraction: logits O(1))
    S[n] = sum_m E[m, n]             (DVE running sum + one PE reduce)
    Z[c, n] = sum_m x2[c, m] E[m, n] (lhsT = host-pretransposed x2)
    M1 = P2 Z ;  out = x1 + gate * relu(M1 * G * (1/S) + Bc)
  with G = gamma*rsqrt(var+eps), Bc = beta + (proj_b + proj_w v_b - mean)*G.
  fusion(j-1) is interleaved into logits(j) on the PE; exp and the softmax
  sum run on ACT/DVE one step behind; 1/S is hidden under the next block.

Everything on the PE runs in float32r (~2e-4 matmul rel err, full rate).
"""
from contextlib import ExitStack

import numpy as np

import concourse.bass as bass
import concourse.mybir as mybir
import concourse.tile as tile
from concourse import bacc
from concourse.bass_utils import run_bass_kernel_spmd

F32 = mybir.dt.float32
F32R = mybir.dt.float32r
AF = mybir.ActivationFunctionType
OP = mybir.AluOpType

B, C, H, W = 4, 256, 64, 64
N = H * W            # 4096
NCORES = 8
NH = N // 2          # 2048 queries per core
NBLK = 512           # query block
NBLOCKS = NH // NBLK
MT = N // 128        # 32 m-tiles
EPS = 1e-5
SCALE = float(C) ** -0.5


def build():
    nc = bacc.Bacc("TRN2", target_bir_lowering=False, debug=False,
                   num_devices=NCORES)
    x1r_d = nc.dram_tensor("x1r", [C, NH], F32R, kind="ExternalInput")
    x2r_d = nc.dram_tensor("x2r", [C, N], F32R, kind="ExternalInput")
    x2t_d = nc.dram_tensor("x2t", [128, MT * C], F32R, kind="ExternalInput")
    wm_d = nc.dram_tensor("wmat", [C, 2 * C], F32R, kind="ExternalInput")
    gw_d = nc.dram_tensor("gw", [C, 2], F32R, kind="ExternalInput")
    vec_d = nc.dram_tensor("vecs", [C, 4], F32, kind="ExternalInput")
    gb_d = nc.dram_tensor("gateb", [1, 1], F32, kind="ExternalInput")
    out_d = nc.dram_tensor("out", [C, NH], F32, kind="ExternalOutput")

    with tile.TileContext(nc) as tc, ExitStack() as ctx:
        pers = ctx.enter_context(tc.tile_pool(name="pers", bufs=1))
        work = ctx.enter_context(tc.tile_pool(name="work", bufs=2))
        psum = ctx.enter_context(tc.tile_pool(name="psum", bufs=1, space="PSUM"))

        # ---- persistent tiles ----
        wm = [pers.tile([128, 2 * C], F32R, tag=f"wm{ci}", name=f"wm{ci}") for ci in range(2)]
        gw = [pers.tile([128, 2], F32R, tag=f"gw{ci}", name=f"gw{ci}") for ci in range(2)]
        vec = [pers.tile([128, 4], F32, tag=f"vec{ci}", name=f"vec{ci}") for ci in range(2)]
        gb = pers.tile([1, 1], F32, tag="gb", name="gb")
        x2r = [pers.tile([128, N], F32R, tag=f"x2r{ci}", name=f"x2r{ci}") for ci in range(2)]
        x2t = pers.tile([128, MT * C], F32R, tag="x2t", name="x2t")
        Qt = [pers.tile([128, NH], F32R, tag=f"Qt{co}", name=f"Qt{co}") for co in range(2)]
        grow = pers.tile([1, NH], F32R, tag="grow", name="grow")
        ones_f = pers.tile([128, 1], F32, tag="ones_f", name="ones_f")
        ones_f2 = pers.tile([1, 128], F32, tag="ones_f2", name="ones_f2")
        ones_c = pers.tile([128, 1], F32R, tag="ones_c", name="ones_c")
        ones_k1 = pers.tile([1, 128], F32R, tag="ones_k1", name="ones_k1")

        # E pool created before xin so both coexist (budgeted); xin's
        # release after gate frees its space for good.
        epool = ctx.enter_context(tc.tile_pool(name="epool", bufs=1))
        E = epool.tile([128, MT * NBLK], F32R, tag="E", name="E")

        def fusion_mms(fp, mt):
            es = slice(mt * NBLK, (mt + 1) * NBLK)
            for co in range(2):
                nc.tensor.matmul(
                    fp[co][:], x2t[:, mt * C + co * 128: mt * C + (co + 1) * 128],
                    E[:, es], start=(mt == 0), stop=(mt == MT - 1))

        def sacc_adds(sacc, mt2):
            e0 = slice((2 * mt2) * NBLK, (2 * mt2 + 1) * NBLK)
            e1 = slice((2 * mt2 + 1) * NBLK, (2 * mt2 + 2) * NBLK)
            if mt2 == 0:
                nc.vector.tensor_add(sacc[:], E[:, e0], E[:, e1])
            else:
                nc.vector.tensor_add(sacc[:], sacc[:], E[:, e0])
                nc.vector.tensor_add(sacc[:], sacc[:], E[:, e1])

        def s_finalize(j, sacc):
            with nc.named_scope(f"sfin{j}"):
                sp = psum.tile([1, NBLK], F32, tag="s", name="s", bufs=1)
                nc.tensor.matmul(sp[:], ones_c[:], sacc[:])
                invs_f = work.tile([1, NBLK], F32, tag="invs_f", name="invs_f",
                                   bufs=1)
                nc.vector.reciprocal_approx_fast(invs_f[:], sp[:])
                invs_r = work.tile([1, NBLK], F32R, tag="invs_r", name="invs_r",
                                   bufs=1)
                nc.vector.tensor_copy(invs_r[:], invs_f[:])
            return invs_r

        def post_block(j, fp, invs_r):
            ns = slice(j * NBLK, (j + 1) * NBLK)
            with nc.named_scope(f"post{j}"):
                Fs = [work.tile([128, NBLK], F32R, tag=f"Fs{co}", name=f"Fs{co}",
                                bufs=1) for co in range(2)]
                for co in range(2):
                    nc.scalar.activation(Fs[co][:], fp[co][:], AF.Copy)
                bc1 = psum.tile([128, NBLK], F32, tag="acc", name="acc", bufs=3)
                nc.tensor.matmul(bc1[:], ones_k1[:], invs_r[:])
                invs_b = work.tile([128, NBLK], F32, tag="invs_b", name="invs_b",
                                   bufs=1)
                nc.vector.tensor_copy(invs_b[:], bc1[:])
                bc2 = psum.tile([128, NBLK], F32, tag="acc", name="acc", bufs=3)
                nc.tensor.matmul(bc2[:], ones_k1[:], grow[:, ns])
                gate_b = work.tile([128, NBLK], F32, tag="gate_b", name="gate_b",
                                   bufs=1)
                nc.vector.tensor_copy(gate_b[:], bc2[:])
                for co in range(2):
                    cs = slice(co * 128, (co + 1) * 128)
                    mp = psum.tile([128, NBLK], F32, tag="acc", name="acc", bufs=3)
                    for ci in range(2):
                        nc.tensor.matmul(
                            mp[:], wm[ci][:, C + co * 128: C + (co + 1) * 128],
                            Fs[ci][:], start=(ci == 0), stop=(ci == 1))
                    x1t = work.tile([128, NBLK], F32R, tag="x1t", name="x1t")
                    nc.sync.dma_start(x1t[:], x1r_d[cs, ns])
                    t1 = work.tile([128, NBLK], F32, tag="t1", name="t1")
                    nc.vector.scalar_tensor_tensor(
                        t1[:], mp[:], vec[co][:, 1:2], invs_b[:],
                        op0=OP.mult, op1=OP.mult)
                    r = work.tile([128, NBLK], F32, tag="r", name="r")
                    nc.scalar.activation(r[:], t1[:], AF.Relu,
                                         bias=vec[co][:, 2:3])
                    rg = work.tile([128, NBLK], F32, tag="t1", name="rg")
                    nc.gpsimd.tensor_mul(rg[:], r[:], gate_b[:])
                    ot = work.tile([128, NBLK], F32, tag="ot", name="ot")
                    nc.gpsimd.tensor_add(ot[:], rg[:], x1t[:].bitcast(F32))
                    nc.sync.dma_start(out_d[cs, ns], ot[:])

        def emit_block(blk, prev_fp, sacc):
            ns = slice(blk * NBLK, (blk + 1) * NBLK)
            for mt2 in range(MT // 2):
                lp = psum.tile([128, 2 * NBLK], F32, tag="L", name="L", bufs=2)
                for sub in range(2):
                    mt = 2 * mt2 + sub
                    msl = slice(mt * 128, (mt + 1) * 128)
                    for ci in range(2):
                        nc.tensor.matmul(
                            lp[:, sub * NBLK:(sub + 1) * NBLK],
                            x2r[ci][:, msl], Qt[ci][:, ns],
                            start=(ci == 0), stop=(ci == 1))
                if prev_fp is not None:
                    fusion_mms(prev_fp, 2 * mt2)
                    fusion_mms(prev_fp, 2 * mt2 + 1)
                nc.scalar.activation(
                    E[:, mt2 * 2 * NBLK:(mt2 + 1) * 2 * NBLK], lp[:],
                    AF.Exp, scale=SCALE)
                if mt2 > 0:
                    sacc_adds(sacc, mt2 - 1)
            sacc_adds(sacc, MT // 2 - 1)

        with nc.named_scope("pre"):
            nc.sync.dma_start(wm[0][:], wm_d[0:128, :])
            nc.gpsimd.dma_start(wm[1][:], wm_d[128:256, :])
            nc.vector.memset(ones_f[:], 1.0)
            nc.vector.tensor_copy(ones_c[:], ones_f[:])
            nc.vector.memset(ones_f2[:], 1.0)
            nc.vector.tensor_copy(ones_k1[:], ones_f2[:])

        sacc0 = None
        with tc.tile_pool(name="xin", bufs=1) as xin:
            x1r = [xin.tile([128, NH], F32R, tag=f"x1r{ci}", name=f"x1r{ci}") for ci in range(2)]
            with nc.named_scope("pre"):
                CH = 1024
                # interleave x1/x2 chunks: Q' and logits0 stream against arrivals
                nc.sync.dma_start(x1r[0][:, 0:CH], x1r_d[0:128, 0:CH])
                nc.gpsimd.dma_start(x1r[1][:, 0:CH], x1r_d[128:256, 0:CH])
                nc.sync.dma_start(x2r[0][:, 0:CH], x2r_d[0:128, 0:CH])
                nc.gpsimd.dma_start(x2r[1][:, 0:CH], x2r_d[128:256, 0:CH])
                nc.sync.dma_start(x1r[0][:, CH:NH], x1r_d[0:128, CH:NH])
                nc.gpsimd.dma_start(x1r[1][:, CH:NH], x1r_d[128:256, CH:NH])
                for ch in range(1, N // CH):
                    chs = slice(ch * CH, (ch + 1) * CH)
                    nc.sync.dma_start(x2r[0][:, chs], x2r_d[0:128, chs])
                    nc.gpsimd.dma_start(x2r[1][:, chs], x2r_d[128:256, chs])
                for ci in range(2):
                    cs = slice(ci * 128, (ci + 1) * 128)
                    nc.sync.dma_start(gw[ci][:], gw_d[cs, :])
                    nc.sync.dma_start(vec[ci][:], vec_d[cs, :])
                nc.sync.dma_start(gb[:], gb_d[:])
                nc.sync.dma_start(x2t[:, 0: MT * C // 2], x2t_d[:, 0: MT * C // 2])
                nc.gpsimd.dma_start(x2t[:, MT * C // 2:], x2t_d[:, MT * C // 2:])

                # Q' projection
                for co in range(2):
                    for nch in range(NH // NBLK):
                        ns = slice(nch * NBLK, (nch + 1) * NBLK)
                        qp = psum.tile([128, NBLK], F32, tag="acc", name="acc", bufs=3)
                        for ci in range(2):
                            nc.tensor.matmul(
                                qp[:], wm[ci][:, co * 128:(co + 1) * 128],
                                x1r[ci][:, ns], start=(ci == 0), stop=(ci == 1))
                        nc.scalar.activation(Qt[co][:, ns], qp[:], AF.Identity,
                                             bias=vec[co][:, 0:1])
            with nc.named_scope("blk0"):
                sacc0 = work.tile([128, NBLK], F32R, tag="sacc", name="sacc",
                                  bufs=2)
                emit_block(0, None, sacc0)
            with nc.named_scope("gate"):
                # gate row (x2 columns pre-permuted: query pixels = 0..NH)
                for blk in range(NBLOCKS):
                    ns = slice(blk * NBLK, (blk + 1) * NBLK)
                    gp = psum.tile([1, NBLK], F32, tag="L", name="gp", bufs=2)
                    for ci in range(2):
                        nc.tensor.matmul(gp[:], gw[ci][:, 0:1], x1r[ci][:, ns],
                                         start=(ci == 0), stop=False)
                    for ci in range(2):
                        nc.tensor.matmul(gp[:], gw[ci][:, 1:2], x2r[ci][:, ns],
                                         start=False, stop=(ci == 1))
                    nc.scalar.activation(grow[:, ns], gp[:], AF.Sigmoid,
                                         bias=gb[:])

        prev_fp = None
        prev_sacc = sacc0
        prev_invs = None
        prev = 0
        for blk in range(1, NBLOCKS):
            with nc.named_scope(f"blk{blk}"):
                prev_invs = s_finalize(prev, prev_sacc)
                prev_fp = [psum.tile([128, NBLK], F32, tag="acc", name="acc",
                                     bufs=3) for _ in range(2)]
                sacc = work.tile([128, NBLK], F32R, tag="sacc", name="sacc",
                                 bufs=2)
                emit_block(blk, prev_fp, sacc)
            post_block(prev, prev_fp, prev_invs)
            prev = blk
            prev_sacc = sacc
        with nc.named_scope("tail"):
            prev_invs = s_finalize(prev, prev_sacc)
            prev_fp = [psum.tile([128, NBLK], F32, tag="acc", name="acc", bufs=3)
                       for _ in range(2)]
            for mt in range(MT):
                fusion_mms(prev_fp, mt)
        post_block(prev, prev_fp, prev_invs)
    nc.compile()
    return nc


_NC = None


def _get_nc():
    global _NC
    if _NC is None:
        _NC = build()
    return _NC


def kernel(**inputs):
    x1 = np.ascontiguousarray(np.asarray(inputs["x1"], dtype=np.float32)).reshape(B, C, N)
    x2 = np.ascontiguousarray(np.asarray(inputs["x2"], dtype=np.float32)).reshape(B, C, N)
    q_w = np.asarray(inputs["q_w"], np.float64)
    k_w = np.asarray(inputs["k_w"], np.float64)
    v_w = np.asarray(inputs["v_w"], np.float64)
    p_w = np.asarray(inputs["proj_w"], np.float64)
    q_b = np.asarray(inputs["q_b"], np.float64)
    v_b = np.asarray(inputs["v_b"], np.float64)
    p_b = np.asarray(inputs["proj_b"], np.float64)
    gamma = np.asarray(inputs["bn_gamma"], np.float64)
    beta = np.asarray(inputs["bn_beta"], np.float64)
    mean = np.asarray(inputs["bn_mean"], np.float64)
    var = np.asarray(inputs["bn_var"], np.float64)
    gate_w = np.asarray(inputs["gate_w"], np.float32)
    gate_b = np.asarray(inputs["gate_b"], np.float32)

    # folded weights: Q' = (k_w^T q_w) x1 + k_w^T q_b ;  M1 = (proj_w v_w) Z
    wqkT = (q_w.T @ k_w).astype(np.float32)          # lhsT for Q' projection
    p2T = (v_w.T @ p_w.T).astype(np.float32)         # lhsT for proj stage
    wmat = np.ascontiguousarray(np.concatenate([wqkT, p2T], axis=1))
    gw = np.ascontiguousarray(
        np.stack([gate_w[0, :C], gate_w[0, C:]], axis=1).astype(np.float32))
    G = gamma / np.sqrt(var + EPS)
    Bc = beta + (p_b + p_w @ v_b - mean) * G
    qpb = k_w.T @ q_b
    vecs = np.ascontiguousarray(
        np.stack([qpb, G, Bc, np.zeros(C)], axis=1).astype(np.float32))
    gb = gate_b.reshape(1, 1)

    in_maps = []
    for core in range(NCORES):
        b, half = divmod(core, 2)
        hq = slice(half * NH, (half + 1) * NH)
        ho = slice((1 - half) * NH, (2 - half) * NH)
        x1q = np.ascontiguousarray(x1[b][:, hq])
        x2p = np.ascontiguousarray(np.concatenate([x2[b][:, hq], x2[b][:, ho]],
                                                  axis=1))
        # x2 pretransposed into the fusion lhsT SBUF layout:
        # x2t[p, mt*C + c] = x2p[c, mt*128 + p]
        x2t = np.ascontiguousarray(
            x2p.reshape(C, MT, 128).transpose(2, 1, 0).reshape(128, MT * C))
        in_maps.append({
            "x1r": x1q, "x2r": x2p, "x2t": x2t,
            "wmat": wmat, "gw": gw, "vecs": vecs, "gateb": gb,
        })

    nc = _get_nc()
    res = run_bass_kernel_spmd(nc, in_maps, core_ids=list(range(NCORES)))
    out = np.empty((B, C, N), np.float32)
    for core in range(NCORES):
        b, half = divmod(core, 2)
        out[b, :, half * NH:(half + 1) * NH] = res.results[core]["out"]
    return out.reshape(B, C, H, W)



# revision 3
# speedup vs baseline: 1.3514x; 1.3514x over previous
"""CrossAttentionFusion Trainium2 kernel — fp8 DoubleRow edition.

Full inputs -> shard (batch x query-half) over 8 NeuronCores -> full output.

Per core (batch b = core//2, query half = core%2): NH=2048 queries n,
N=4096 keys m, C=256 channels.

Host precompute (exact f32, then fp8e4 quantization):
  Q'[c,n] = (q_w^T k_w)^T x1 + k_w^T q_b        (logits rhs)
  Y[o,m]  = G[o] * (p_w v_w x2)[o,m]            (fusion rhs; G = BN scale)
  gate[n] = sigmoid(gate_w [x1;x2] + gate_b)    (per-query scalar)
  Bc[o]   = beta + (p_b + p_w v_b - mean) * G   (post bias row)
Device per 512-query block j:
  L[m,n]  = x2^T Q'      fp8 DoubleRow matmuls (contraction c=256/instr)
  E       = exp(L/16 - 2.5) on ACT, fp8e4 out (offset cancels in Z/S)
  P[n,o]  = sum_m E[m,n] Y[o,m]  fp8 DR, out in [query, channel] layout;
            Y carries a ones column so P[:,256] = S = softmax denominator
  out^T   = x1^T + gate * relu(P[:, :256]/S + Bc)   (DVE recip/STT,
            GpSimd relu*gate, DVE residual add; all per-partition scalars)
  fusion(j-1) instrs interleave into logits(j) slots on the PE; exp is the
  pacing engine (~55us floor: 8.4M elements at 128 lanes / 1.2 GHz).
"""
from contextlib import ExitStack

import numpy as np
import ml_dtypes

import concourse.bass as bass
import concourse.mybir as mybir
import concourse.tile as tile
from concourse import bacc
from concourse.bass_utils import run_bass_kernel_spmd

F32 = mybir.dt.float32
FP8 = mybir.dt.float8e4
AF = mybir.ActivationFunctionType
OP = mybir.AluOpType
DR = mybir.MatmulPerfMode.DoubleRow
NP8 = ml_dtypes.float8_e4m3

B, C, H, W = 4, 256, 64, 64
N = H * W            # 4096 keys per batch
NCORES = 8
NH = N // 2          # 2048 queries per core
NBLK = 512           # query block
NBLOCKS = NH // NBLK
MT = N // 128        # 32 key tiles
MT2 = MT // 2        # 16 DoubleRow key-pair steps
YW = 272             # yt row: 256 channels + ones col + pad to %16
NT = NBLK // 128     # 4 query tiles per block
EPS = 1e-5
SCALE = float(C) ** -0.5
EOFF = 2.5           # exp offset; cancels in Z/S


def build():
    nc = bacc.Bacc("TRN2", target_bir_lowering=False, debug=False,
                   num_devices=NCORES)
    q8_d = nc.dram_tensor("q8", [128, 2 * NH], FP8, kind="ExternalInput")
    x2_d = nc.dram_tensor("x2dr", [128, 2 * N], FP8, kind="ExternalInput")
    yt_d = nc.dram_tensor("yt", [128, MT * YW], FP8, kind="ExternalInput")
    x1_d = nc.dram_tensor("x1t", [NH, C], F32, kind="ExternalInput")
    gc_d = nc.dram_tensor("gatec", [128, NBLOCKS * NT], F32,
                          kind="ExternalInput")
    bc_d = nc.dram_tensor("bct", [128, C], F32, kind="ExternalInput")
    out_d = nc.dram_tensor("out", [NH, C], F32, kind="ExternalOutput")

    with tile.TileContext(nc) as tc, ExitStack() as ctx:
        pers = ctx.enter_context(tc.tile_pool(name="pers", bufs=1))
        epool = ctx.enter_context(tc.tile_pool(name="epool", bufs=2))
        work = ctx.enter_context(tc.tile_pool(name="work", bufs=2))
        outs = ctx.enter_context(tc.tile_pool(name="outs", bufs=3))
        psL = ctx.enter_context(tc.tile_pool(name="psL", bufs=2, space="PSUM"))
        psF = ctx.enter_context(tc.tile_pool(name="psF", bufs=4, space="PSUM"))

        q8 = pers.tile([128, 2, NH], FP8, tag="q8", name="q8")
        x2dr = pers.tile([128, 2, N], FP8, tag="x2dr", name="x2dr")
        yt = pers.tile([128, MT, YW], FP8, tag="yt", name="yt")
        x1p = pers.tile([128, NBLOCKS * NT, C], F32, tag="x1p", name="x1p")
        bct = pers.tile([128, C], F32, tag="bct", name="bct")
        gc = pers.tile([128, NBLOCKS * NT], F32, tag="gc", name="gc")
        nbias = pers.tile([128, 1], F32, tag="nbias", name="nbias")

        with nc.named_scope("pre"):
            nc.vector.memset(nbias[:], -EOFF)
            # q8 first (block 0 rhs), then x2dr in column chunks so early
            # key-tiles land first; yt/bct/gc/x1p are needed a block later.
            nc.sync.dma_start(q8[:, 0, :], q8_d[:, 0:NH])
            nc.gpsimd.dma_start(q8[:, 1, :], q8_d[:, NH:2 * NH])
            CH = 1024
            for c0 in range(0, N, CH):
                nc.sync.dma_start(x2dr[:, 0, c0:c0 + CH], x2_d[:, c0:c0 + CH])
                nc.gpsimd.dma_start(x2dr[:, 1, c0:c0 + CH],
                                    x2_d[:, N + c0:N + c0 + CH])
            nc.sync.dma_start(yt[:, 0:MT2, :],
                              yt_d[:, 0:MT2 * YW].rearrange(
                                  "p (t y) -> p t y", y=YW))
            nc.gpsimd.dma_start(yt[:, MT2:MT, :],
                                yt_d[:, MT2 * YW:].rearrange(
                                    "p (t y) -> p t y", y=YW))
            nc.sync.dma_start(bct[:], bc_d[:])
            nc.sync.dma_start(gc[:], gc_d[:])
            for t in range(NBLOCKS * NT):
                eng = nc.sync if t % 2 == 0 else nc.gpsimd
                eng.dma_start(x1p[:, t, :], x1_d[t * 128:(t + 1) * 128, :])

        def emit_fusion(e8, nt, k, fuse):
            if k == 0:
                fuse[nt] = psF.tile([128, 257], F32, tag="fuse", name="fuse")
            nc.tensor.matmul(
                fuse[nt][:, 0:257],
                e8[:, 2 * k:2 * k + 2, nt * 128:(nt + 1) * 128],
                yt[:, 2 * k:2 * k + 2, 0:257],
                start=(k == 0), stop=(k == MT2 - 1), perf_mode=DR)

        def emit_post(j, nt, fuse):
            t_idx = j * NT + nt
            with nc.named_scope(f"post{j}_{nt}"):
                fp = fuse[nt]
                invs = work.tile([128, 1], F32, tag="invs", name="invs")
                nc.vector.reciprocal_approx_fast(invs[:], fp[:, 256:257])
                tt = work.tile([128, C], F32, tag="tt", name="tt")
                nc.vector.scalar_tensor_tensor(
                    tt[:], fp[:, 0:256], invs[:], bct[:],
                    op0=OP.mult, op1=OP.add)
                rg = work.tile([128, C], F32, tag="rg", name="rg")
                nc.gpsimd.tensor_scalar(
                    rg[:], tt[:], 0.0, gc[:, t_idx:t_idx + 1],
                    op0=OP.max, op1=OP.mult)
                ot = outs.tile([128, C], F32, tag="ot", name="ot")
                nc.vector.tensor_add(ot[:], rg[:], x1p[:, t_idx, :])
                nc.sync.dma_start(out_d[t_idx * 128:(t_idx + 1) * 128, :],
                                  ot[:])

        prev_e8 = None
        for j in range(NBLOCKS):
            ns = slice(j * NBLK, (j + 1) * NBLK)
            with nc.named_scope(f"blk{j}"):
                e8 = epool.tile([128, MT, NBLK], FP8, tag="E8", name="E8")
                fuse = {}
                fcount = 0
                for mt2 in range(MT2):
                    lp = psL.tile([128, 2, NBLK], F32, tag="L", name="L")
                    for sub in range(2):
                        mt = 2 * mt2 + sub
                        nc.tensor.matmul(
                            lp[:, sub, :],
                            x2dr[:, :, mt * 128:(mt + 1) * 128],
                            q8[:, :, ns], start=True, stop=True, perf_mode=DR)
                    if prev_e8 is not None:
                        for _ in range(4):
                            nt, k = divmod(fcount, MT2)
                            emit_fusion(prev_e8, nt, k, fuse)
                            fcount += 1
                            if k == MT2 - 1:
                                emit_post(j - 1, nt, fuse)
                    nc.scalar.activation(e8[:, 2 * mt2:2 * mt2 + 2, :], lp[:],
                                         AF.Exp, scale=SCALE, bias=nbias[:])
                prev_e8 = e8
        with nc.named_scope("tail"):
            fuse = {}
            for f in range(NT * MT2):
                nt, k = divmod(f, MT2)
                emit_fusion(prev_e8, nt, k, fuse)
                if k == MT2 - 1:
                    emit_post(NBLOCKS - 1, nt, fuse)
    nc.compile()
    return nc


_NC = None


def _get_nc():
    global _NC
    if _NC is None:
        _NC = build()
    return _NC


def kernel(**inputs):
    x1 = np.asarray(inputs["x1"], np.float32).reshape(B, C, N)
    x2 = np.asarray(inputs["x2"], np.float32).reshape(B, C, N)
    q_w = np.asarray(inputs["q_w"], np.float32)
    k_w = np.asarray(inputs["k_w"], np.float32)
    v_w = np.asarray(inputs["v_w"], np.float32)
    p_w = np.asarray(inputs["proj_w"], np.float32)
    q_b = np.asarray(inputs["q_b"], np.float32)
    v_b = np.asarray(inputs["v_b"], np.float32)
    p_b = np.asarray(inputs["proj_b"], np.float32)
    gamma = np.asarray(inputs["bn_gamma"], np.float32)
    beta = np.asarray(inputs["bn_beta"], np.float32)
    mean = np.asarray(inputs["bn_mean"], np.float32)
    var = np.asarray(inputs["bn_var"], np.float32)
    gate_w = np.asarray(inputs["gate_w"], np.float32)
    gate_b = np.asarray(inputs["gate_b"], np.float32)

    wqk = q_w.T @ k_w                      # [C,C]
    A = gamma[:, None] / np.sqrt(var + EPS)[:, None] * (p_w @ v_w)  # G*(pw vw)
    G = gamma / np.sqrt(var + EPS)
    Bc = (beta + (p_b + p_w @ v_b - mean) * G).astype(np.float32)
    qpb = (k_w.T @ q_b).astype(np.float32)
    bct = np.ascontiguousarray(np.broadcast_to(Bc, (128, C)))

    in_maps = []
    for b in range(B):
        Qp = (wqk.T @ x1[b] + qpb[:, None]).astype(NP8)      # [C, N]
        Y8 = (A @ x2[b]).astype(NP8)                          # [C, N]
        glog = gate_w[0, :C] @ x1[b] + gate_w[0, C:] @ x2[b] + gate_b[0]
        gate = (1.0 / (1.0 + np.exp(-glog))).astype(np.float32)  # [N]
        x28 = x2[b].astype(NP8)
        # x2dr/q8 layouts: [p, h, m] = arr[h*128+p, m]
        x2dr = np.ascontiguousarray(
            x28.reshape(2, 128, N).transpose(1, 0, 2).reshape(128, 2 * N))
        yt = np.zeros((128, MT, YW), NP8)
        yt[:, :, :C] = np.ascontiguousarray(
            Y8.reshape(C, MT, 128).transpose(2, 1, 0))
        yt[:, :, C] = np.float32(1.0)
        yt = np.ascontiguousarray(yt.reshape(128, MT * YW))
        for half in range(2):
            hq = slice(half * NH, (half + 1) * NH)
            q8 = np.ascontiguousarray(
                Qp[:, hq].reshape(2, 128, NH).transpose(1, 0, 2)
                .reshape(128, 2 * NH))
            x1t = np.ascontiguousarray(x1[b][:, hq].T)        # [NH, C] f32
            gc = np.ascontiguousarray(
                gate[hq].reshape(NBLOCKS * NT, 128).T.astype(np.float32))
            in_maps.append({
                "q8": q8, "x2dr": x2dr, "yt": yt, "x1t": x1t,
                "gatec": gc, "bct": bct,
            })

    nc = _get_nc()
    res = run_bass_kernel_spmd(nc, in_maps, core_ids=list(range(NCORES)))
    out = np.empty((B, C, N), np.float32)
    for core in range(NCORES):
        b, half = divmod(core, 2)
        out[b, :, half * NH:(half + 1) * NH] = res.results[core]["out"].T
    return out.reshape(B, C, H, W)


# revision 15
# speedup vs baseline: 1.9239x; 1.4237x over previous
"""CrossAttentionFusion Trainium2 kernel — fp8 DoubleRow edition.

Full inputs -> shard (batch x query-half) over 8 NeuronCores -> full output.

Per core (batch b = core//2, query half = core%2): NH=2048 queries n,
N=4096 keys m, C=256 channels.

Host precompute (exact f32, then fp8e4 quantization):
  Q'[c,n] = (q_w^T k_w)^T x1 + k_w^T q_b        (logits rhs)
  Y[o,m]  = G[o] * (p_w v_w x2)[o,m]            (fusion rhs; G = BN scale)
  gate[n] = sigmoid(gate_w [x1;x2] + gate_b)    (per-query scalar)
  Bc[o]   = beta + (p_b + p_w v_b - mean) * G   (post bias row)
Device per 512-query block j:
  L[m,n]  = x2^T Q'      fp8 DoubleRow matmuls (contraction c=256/instr)
  E       = exp(L/16 - 2.5) on ACT, fp8e4 out (offset cancels in Z/S)
  P[n,o]  = sum_m E[m,n] Y[o,m]  fp8 DR, out in [query, channel] layout;
            Y carries a ones column so P[:,256] = S = softmax denominator
  out^T   = x1^T + gate * relu(P[:, :256]/S + Bc)   (DVE recip/STT,
            GpSimd relu*gate, DVE residual add; all per-partition scalars)
  fusion(j-1) instrs interleave into logits(j) slots on the PE; exp is the
  pacing engine (~55us floor: 8.4M elements at 128 lanes / 1.2 GHz).
"""
from contextlib import ExitStack

import numpy as np
import ml_dtypes

import concourse.bass as bass
import concourse.mybir as mybir
import concourse.tile as tile
from concourse import bacc
from concourse.bass_utils import run_bass_kernel_spmd

F32 = mybir.dt.float32
BF16 = mybir.dt.bfloat16
FP8 = mybir.dt.float8e4
AF = mybir.ActivationFunctionType
OP = mybir.AluOpType
DR = mybir.MatmulPerfMode.DoubleRow
NP8 = ml_dtypes.float8_e4m3

B, C, H, W = 4, 256, 64, 64
N = H * W            # 4096 keys per batch
NCORES = 8
NH = N // 2          # 2048 queries per core
NBLK = 512           # query block
NBLOCKS = NH // NBLK
MT = N // 128        # 32 key tiles
MT2 = MT // 2        # 16 DoubleRow key-pair steps
YW = 272             # yt row: 256 channels + ones col + pad to %16
NT = NBLK // 128     # 4 query tiles per block
EPS = 1e-5
SCALE = float(C) ** -0.5
EOFF = 2.5           # exp offset; cancels in Z/S


def build():
    nc = bacc.Bacc("TRN2", target_bir_lowering=False, debug=False,
                   num_devices=NCORES)
    q8_d = nc.dram_tensor("q8", [128, 2 * NH], FP8, kind="ExternalInput")
    x2_d = nc.dram_tensor("x2dr", [128, 2 * N], FP8, kind="ExternalInput")
    yt_d = nc.dram_tensor("yt", [128, MT * YW], FP8, kind="ExternalInput")
    x1_d = nc.dram_tensor("x1t", [NH, C], BF16, kind="ExternalInput")
    gc_d = nc.dram_tensor("gatec", [128, NBLOCKS * NT], F32,
                          kind="ExternalInput")
    bc_d = nc.dram_tensor("bct", [128, C], F32, kind="ExternalInput")
    out_d = nc.dram_tensor("out", [NH, C], BF16, kind="ExternalOutput")

    with tile.TileContext(nc) as tc, ExitStack() as ctx:
        pers = ctx.enter_context(tc.tile_pool(name="pers", bufs=1))
        epool = ctx.enter_context(tc.tile_pool(name="epool", bufs=2))
        work = ctx.enter_context(tc.tile_pool(name="work", bufs=2))
        outs = ctx.enter_context(tc.tile_pool(name="outs", bufs=3))
        psL = ctx.enter_context(tc.tile_pool(name="psL", bufs=2, space="PSUM"))
        psF = ctx.enter_context(tc.tile_pool(name="psF", bufs=4, space="PSUM"))

        q8 = pers.tile([128, 2, NH], FP8, tag="q8", name="q8")
        x2dr = pers.tile([128, 2, N], FP8, tag="x2dr", name="x2dr")
        yt = pers.tile([128, MT, YW], FP8, tag="yt", name="yt")
        x1p = pers.tile([128, NBLOCKS * NT, C], BF16, tag="x1p", name="x1p")
        bct = pers.tile([128, C], F32, tag="bct", name="bct")
        gc = pers.tile([128, NBLOCKS * NT], F32, tag="gc", name="gc")
        nbias = pers.tile([128, 1], F32, tag="nbias", name="nbias")

        with nc.named_scope("pre"):
            nc.vector.memset(nbias[:], -EOFF)
            # Consumers wait on ALL DMAs issued so far on a queue, so only
            # block-0's first needs go up front; the rest is issued from
            # inside the slot loop just ahead of use (see dma_feed below).
            nc.sync.dma_start(q8[:, 0, :], q8_d[:, 0:NH])
            nc.gpsimd.dma_start(q8[:, 1, :], q8_d[:, NH:2 * NH])
            CH = 1024
            nc.sync.dma_start(x2dr[:, 0, 0:CH], x2_d[:, 0:CH])
            nc.gpsimd.dma_start(x2dr[:, 1, 0:CH], x2_d[:, N:N + CH])

        def _x2(c):
            c0 = c * 1024
            nc.sync.dma_start(x2dr[:, 0, c0:c0 + 1024], x2_d[:, c0:c0 + 1024])
            nc.gpsimd.dma_start(x2dr[:, 1, c0:c0 + 1024],
                                x2_d[:, N + c0:N + c0 + 1024])

        def _yt(t0):
            eng = nc.sync if (t0 // 4) % 2 == 0 else nc.gpsimd
            eng.dma_start(
                yt[:, t0:t0 + 4, :],
                yt_d[:, t0 * YW:(t0 + 4) * YW].rearrange(
                    "p (t y) -> p t y", y=YW))

        def dma_feed():
            # Deferred DMA issues, one step per slot, just ahead of need.
            # Consumers wait on all prior issues of a queue, so issuing late
            # (but before the consuming instr is emitted) is what overlaps
            # transfers with block-0 compute.
            _x2(1); _yt(0); _yt(4)
            yield
            _x2(2)
            yield
            _yt(8); _yt(12)
            yield
            _x2(3)
            yield
            _yt(16); _yt(20)
            yield
            _yt(24); _yt(28)
            yield
            nc.sync.dma_start(bct[:], bc_d[:])
            nc.sync.dma_start(gc[:], gc_d[:])
            yield
            for t in range(NBLOCKS * NT):
                eng = nc.gpsimd if t % 2 == 0 else nc.sync
                eng.dma_start(x1p[:, t, :], x1_d[t * 128:(t + 1) * 128, :])
                if t % 2 == 1:
                    yield

        feed = dma_feed()

        def emit_fusion(e8, nt, k, fuse):
            if k == 0:
                fuse[nt] = psF.tile([128, 257], F32, tag="fuse", name="fuse")
            nc.tensor.matmul(
                fuse[nt][:, 0:257],
                e8[:, 2 * k:2 * k + 2, nt * 128:(nt + 1) * 128],
                yt[:, 2 * k:2 * k + 2, 0:257],
                start=(k == 0), stop=(k == MT2 - 1), perf_mode=DR)

        def emit_post(j, nt, fuse):
            t_idx = j * NT + nt
            with nc.named_scope(f"post{j}_{nt}"):
                fp = fuse[nt]
                invs = work.tile([128, 1], F32, tag="invs", name="invs")
                nc.vector.reciprocal_approx_fast(invs[:], fp[:, 256:257])
                tt = work.tile([128, C], F32, tag="tt", name="tt")
                nc.vector.scalar_tensor_tensor(
                    tt[:], fp[:, 0:256], invs[:], bct[:],
                    op0=OP.mult, op1=OP.add)
                rg = work.tile([128, C], F32, tag="rg", name="rg")
                nc.vector.tensor_scalar_max(rg[:], tt[:], 0.0)
                ot = outs.tile([128, C], BF16, tag="ot", name="ot")
                nc.vector.scalar_tensor_tensor(
                    ot[:], rg[:], gc[:, t_idx:t_idx + 1], x1p[:, t_idx, :],
                    op0=OP.mult, op1=OP.add)
                nc.sync.dma_start(out_d[t_idx * 128:(t_idx + 1) * 128, :],
                                  ot[:])

        # Chasing schedule: fusion(j, k) runs one slot behind exp(j, k); the
        # final k lands in the next block's first slot (or the tail).
        prev = None  # (e8, fuse, j) with k=15 + posts pending
        for j in range(NBLOCKS):
            ns = slice(j * NBLK, (j + 1) * NBLK)
            with nc.named_scope(f"blk{j}"):
                e8 = epool.tile([128, MT, NBLK], FP8, tag="E8", name="E8")
                fuse = {}
                for mt2 in range(MT2):
                    lp = psL.tile([128, 2, NBLK], F32, tag="L", name="L")
                    for sub in range(2):
                        mt = 2 * mt2 + sub
                        nc.tensor.matmul(
                            lp[:, sub, :],
                            x2dr[:, :, mt * 128:(mt + 1) * 128],
                            q8[:, :, ns], start=True, stop=True, perf_mode=DR)
                    if mt2 == 0 and prev is not None:
                        pe8, pfuse, pj = prev
                        for nt in range(NT):
                            emit_fusion(pe8, nt, MT2 - 1, pfuse)
                            emit_post(pj, nt, pfuse)
                        prev = None
                    if mt2 >= 1:
                        for nt in range(NT):
                            emit_fusion(e8, nt, mt2 - 1, fuse)
                    nc.scalar.activation(e8[:, 2 * mt2:2 * mt2 + 2, :], lp[:],
                                         AF.Exp, scale=SCALE, bias=nbias[:])
                    next(feed, None)
                prev = (e8, fuse, j)
        with nc.named_scope("tail"):
            pe8, pfuse, pj = prev
            for nt in range(NT):
                emit_fusion(pe8, nt, MT2 - 1, pfuse)
                emit_post(pj, nt, pfuse)
    nc.compile()
    return nc


_NC = None


def _get_nc():
    global _NC
    if _NC is None:
        _NC = build()
    return _NC


def kernel(**inputs):
    x1 = np.asarray(inputs["x1"], np.float32).reshape(B, C, N)
    x2 = np.asarray(inputs["x2"], np.float32).reshape(B, C, N)
    q_w = np.asarray(inputs["q_w"], np.float32)
    k_w = np.asarray(inputs["k_w"], np.float32)
    v_w = np.asarray(inputs["v_w"], np.float32)
    p_w = np.asarray(inputs["proj_w"], np.float32)
    q_b = np.asarray(inputs["q_b"], np.float32)
    v_b = np.asarray(inputs["v_b"], np.float32)
    p_b = np.asarray(inputs["proj_b"], np.float32)
    gamma = np.asarray(inputs["bn_gamma"], np.float32)
    beta = np.asarray(inputs["bn_beta"], np.float32)
    mean = np.asarray(inputs["bn_mean"], np.float32)
    var = np.asarray(inputs["bn_var"], np.float32)
    gate_w = np.asarray(inputs["gate_w"], np.float32)
    gate_b = np.asarray(inputs["gate_b"], np.float32)

    wqk = q_w.T @ k_w                      # [C,C]
    A = gamma[:, None] / np.sqrt(var + EPS)[:, None] * (p_w @ v_w)  # G*(pw vw)
    G = gamma / np.sqrt(var + EPS)
    Bc = (beta + (p_b + p_w @ v_b - mean) * G).astype(np.float32)
    qpb = (k_w.T @ q_b).astype(np.float32)
    bct = np.ascontiguousarray(np.broadcast_to(Bc, (128, C)))

    in_maps = []
    for b in range(B):
        Qp = (wqk.T @ x1[b] + qpb[:, None]).astype(NP8)      # [C, N]
        Y8 = (A @ x2[b]).astype(NP8)                          # [C, N]
        glog = gate_w[0, :C] @ x1[b] + gate_w[0, C:] @ x2[b] + gate_b[0]
        gate = (1.0 / (1.0 + np.exp(-glog))).astype(np.float32)  # [N]
        x28 = x2[b].astype(NP8)
        # x2dr/q8 layouts: [p, h, m] = arr[h*128+p, m]
        x2dr = np.ascontiguousarray(
            x28.reshape(2, 128, N).transpose(1, 0, 2).reshape(128, 2 * N))
        yt = np.zeros((128, MT, YW), NP8)
        yt[:, :, :C] = np.ascontiguousarray(
            Y8.reshape(C, MT, 128).transpose(2, 1, 0))
        yt[:, :, C] = np.float32(1.0)
        yt = np.ascontiguousarray(yt.reshape(128, MT * YW))
        for half in range(2):
            hq = slice(half * NH, (half + 1) * NH)
            q8 = np.ascontiguousarray(
                Qp[:, hq].reshape(2, 128, NH).transpose(1, 0, 2)
                .reshape(128, 2 * NH))
            x1t = np.ascontiguousarray(
                x1[b][:, hq].T.astype(ml_dtypes.bfloat16))    # [NH, C]
            gc = np.ascontiguousarray(
                gate[hq].reshape(NBLOCKS * NT, 128).T.astype(np.float32))
            in_maps.append({
                "q8": q8, "x2dr": x2dr, "yt": yt, "x1t": x1t,
                "gatec": gc, "bct": bct,
            })

    nc = _get_nc()
    res = run_bass_kernel_spmd(nc, in_maps, core_ids=list(range(NCORES)))
    out = np.empty((B, C, N), np.float32)
    for core in range(NCORES):
        b, half = divmod(core, 2)
        out[b, :, half * NH:(half + 1) * NH] = \
            res.results[core]["out"].astype(np.float32).T
    return out.reshape(B, C, H, W)


# revision 19
# speedup vs baseline: 1.9296x; 1.0029x over previous
"""CrossAttentionFusion Trainium2 kernel — fp8 DoubleRow edition.

Full inputs -> shard (batch x query-half) over 8 NeuronCores -> full output.

Per core (batch b = core//2, query half = core%2): NH=2048 queries n,
N=4096 keys m, C=256 channels.

Host precompute (exact f32, then fp8e4 quantization):
  Q'[c,n] = (q_w^T k_w)^T x1 + k_w^T q_b        (logits rhs)
  Y[o,m]  = G[o] * (p_w v_w x2)[o,m]            (fusion rhs; G = BN scale)
  gate[n] = sigmoid(gate_w [x1;x2] + gate_b)    (per-query scalar)
  Bc[o]   = beta + (p_b + p_w v_b - mean) * G   (post bias row)
Device per 512-query block j:
  L[m,n]  = x2^T Q'      fp8 DoubleRow matmuls (contraction c=256/instr)
  E       = exp(L/16 - 2.5) on ACT, fp8e4 out (offset cancels in Z/S)
  P[n,o]  = sum_m E[m,n] Y[o,m]  fp8 DR, out in [query, channel] layout;
            Y carries a ones column so P[:,256] = S = softmax denominator
  out^T   = x1^T + gate * relu(P[:, :256]/S + Bc)   (DVE recip/STT,
            GpSimd relu*gate, DVE residual add; all per-partition scalars)
  fusion(j-1) instrs interleave into logits(j) slots on the PE; exp is the
  pacing engine (~55us floor: 8.4M elements at 128 lanes / 1.2 GHz).
"""
from contextlib import ExitStack

import numpy as np
import ml_dtypes

import concourse.bass as bass
import concourse.mybir as mybir
import concourse.tile as tile
from concourse import bacc
from concourse.bass_utils import run_bass_kernel_spmd

F32 = mybir.dt.float32
BF16 = mybir.dt.bfloat16
FP8 = mybir.dt.float8e4
AF = mybir.ActivationFunctionType
OP = mybir.AluOpType
DR = mybir.MatmulPerfMode.DoubleRow
NP8 = ml_dtypes.float8_e4m3

B, C, H, W = 4, 256, 64, 64
N = H * W            # 4096 keys per batch
NCORES = 8
NH = N // 2          # 2048 queries per core
NBLK = 512           # query block
NBLOCKS = NH // NBLK
MT = N // 128        # 32 key tiles
MT2 = MT // 2        # 16 DoubleRow key-pair steps
YW = 272             # yt row: 256 channels + ones col + pad to %16
NT = NBLK // 128     # 4 query tiles per block
EPS = 1e-5
SCALE = float(C) ** -0.5
EOFF = 2.5           # exp offset; cancels in Z/S


def build():
    nc = bacc.Bacc("TRN2", target_bir_lowering=False, debug=False,
                   num_devices=NCORES)
    q8_d = nc.dram_tensor("q8", [128, 2 * NH], FP8, kind="ExternalInput")
    x2_d = nc.dram_tensor("x2dr", [128, 2 * N], FP8, kind="ExternalInput")
    yt_d = nc.dram_tensor("yt", [128, MT * YW], FP8, kind="ExternalInput")
    x1_d = nc.dram_tensor("x1t", [NH, C], BF16, kind="ExternalInput")
    gc_d = nc.dram_tensor("gatec", [128, NBLOCKS * NT], F32,
                          kind="ExternalInput")
    bc_d = nc.dram_tensor("bct", [128, C], F32, kind="ExternalInput")
    out_d = nc.dram_tensor("out", [NH, C], BF16, kind="ExternalOutput")

    with tile.TileContext(nc) as tc, ExitStack() as ctx:
        pers = ctx.enter_context(tc.tile_pool(name="pers", bufs=1))
        epool = ctx.enter_context(tc.tile_pool(name="epool", bufs=2))
        work = ctx.enter_context(tc.tile_pool(name="work", bufs=2))
        outs = ctx.enter_context(tc.tile_pool(name="outs", bufs=3))
        psL = ctx.enter_context(tc.tile_pool(name="psL", bufs=2, space="PSUM"))
        psF = ctx.enter_context(tc.tile_pool(name="psF", bufs=4, space="PSUM"))

        q8 = pers.tile([128, 2, NH], FP8, tag="q8", name="q8")
        # chunked tiles: dependency tracking is whole-tile, so chunk tiles
        # let block-0 compute start as soon as its chunk lands
        x2c = [pers.tile([128, 2, 1024], FP8, tag=f"x2c{c}", name=f"x2c{c}")
               for c in range(4)]
        ytc = [pers.tile([128, 4, YW], FP8, tag=f"ytc{i}", name=f"ytc{i}")
               for i in range(8)]
        x1p = pers.tile([128, NBLOCKS * NT, C], BF16, tag="x1p", name="x1p")
        bct = pers.tile([128, C], F32, tag="bct", name="bct")
        gc = pers.tile([128, NBLOCKS * NT], F32, tag="gc", name="gc")
        nbias = pers.tile([128, 1], F32, tag="nbias", name="nbias")

        def _x2(c):
            c0 = c * 1024
            nc.sync.dma_start(x2c[c][:, 0, :], x2_d[:, c0:c0 + 1024])
            nc.gpsimd.dma_start(x2c[c][:, 1, :],
                                x2_d[:, N + c0:N + c0 + 1024])

        def _yt(t0):
            eng = nc.sync if (t0 // 4) % 2 == 0 else nc.gpsimd
            eng.dma_start(
                ytc[t0 // 4][:],
                yt_d[:, t0 * YW:(t0 + 4) * YW].rearrange(
                    "p (t y) -> p t y", y=YW))

        with nc.named_scope("pre"):
            nc.vector.memset(nbias[:], -EOFF)
            # PE warmup: ramp the clock out of the low p-state on scratch
            # data while the first DMAs land.
            wl = pers.tile([128, 2, 128], FP8, tag="wl", name="wl")
            wr = pers.tile([128, 2, 257], FP8, tag="wr", name="wr")
            nc.vector.memset(wl[:], 0.0)
            nc.vector.memset(wr[:], 0.0)
            for w in range(6):
                wp = psF.tile([128, 257], F32, tag="fuse", name="fuse")
                nc.tensor.matmul(wp[:], wl[:], wr[:], start=True, stop=True,
                                 perf_mode=DR)
            nc.sync.dma_start(q8[:, 0, :], q8_d[:, 0:NH])
            nc.gpsimd.dma_start(q8[:, 1, :], q8_d[:, NH:2 * NH])
            _x2(0)

        def dma_feed():
            # Deferred DMA issues, one step per slot, just ahead of need.
            # Consumers wait on all prior issues of a queue, so issuing late
            # (but before the consuming instr is emitted) is what overlaps
            # transfers with block-0 compute.
            _x2(1); _yt(0); _yt(4)
            yield
            _x2(2)
            yield
            _yt(8); _yt(12)
            yield
            _x2(3)
            yield
            _yt(16); _yt(20)
            yield
            _yt(24); _yt(28)
            yield
            nc.sync.dma_start(bct[:], bc_d[:])
            nc.sync.dma_start(gc[:], gc_d[:])
            yield
            for t in range(NBLOCKS * NT):
                eng = nc.gpsimd if t % 2 == 0 else nc.sync
                eng.dma_start(x1p[:, t, :], x1_d[t * 128:(t + 1) * 128, :])
                if t % 2 == 1:
                    yield

        feed = dma_feed()

        def emit_fusion(e8, nt, k, fuse):
            if k == 0:
                fuse[nt] = psF.tile([128, 257], F32, tag="fuse", name="fuse")
            sub = (2 * k) % 4
            nc.tensor.matmul(
                fuse[nt][:, 0:257],
                e8[:, 2 * k:2 * k + 2, nt * 128:(nt + 1) * 128],
                ytc[k // 2][:, sub:sub + 2, 0:257],
                start=(k == 0), stop=(k == MT2 - 1), perf_mode=DR)

        def emit_post(j, nt, fuse):
            t_idx = j * NT + nt
            with nc.named_scope(f"post{j}_{nt}"):
                fp = fuse[nt]
                invs = work.tile([128, 1], F32, tag="invs", name="invs")
                nc.vector.reciprocal_approx_fast(invs[:], fp[:, 256:257])
                tt = work.tile([128, C], F32, tag="tt", name="tt")
                nc.vector.scalar_tensor_tensor(
                    tt[:], fp[:, 0:256], invs[:], bct[:],
                    op0=OP.mult, op1=OP.add)
                rg = work.tile([128, C], F32, tag="rg", name="rg")
                nc.vector.tensor_scalar_max(rg[:], tt[:], 0.0)
                ot = outs.tile([128, C], BF16, tag="ot", name="ot")
                nc.vector.scalar_tensor_tensor(
                    ot[:], rg[:], gc[:, t_idx:t_idx + 1], x1p[:, t_idx, :],
                    op0=OP.mult, op1=OP.add)
                nc.sync.dma_start(out_d[t_idx * 128:(t_idx + 1) * 128, :],
                                  ot[:])

        # Chasing schedule: fusion(j, k) runs one slot behind exp(j, k); the
        # final k lands in the next block's first slot (or the tail).
        prev = None  # (e8, fuse, j) with k=15 + posts pending
        for j in range(NBLOCKS):
            ns = slice(j * NBLK, (j + 1) * NBLK)
            with nc.named_scope(f"blk{j}"):
                e8 = epool.tile([128, MT, NBLK], FP8, tag="E8", name="E8")
                fuse = {}
                for mt2 in range(MT2):
                    lp = psL.tile([128, 2, NBLK], F32, tag="L", name="L")
                    for sub in range(2):
                        mt = 2 * mt2 + sub
                        cc = (mt % 8) * 128
                        nc.tensor.matmul(
                            lp[:, sub, :],
                            x2c[mt // 8][:, :, cc:cc + 128],
                            q8[:, :, ns], start=True, stop=True, perf_mode=DR)
                    if mt2 == 0 and prev is not None:
                        pe8, pfuse, pj = prev
                        for nt in range(NT):
                            emit_fusion(pe8, nt, MT2 - 1, pfuse)
                            emit_post(pj, nt, pfuse)
                        prev = None
                    if mt2 >= 1:
                        for nt in range(NT):
                            emit_fusion(e8, nt, mt2 - 1, fuse)
                    nc.scalar.activation(e8[:, 2 * mt2:2 * mt2 + 2, :], lp[:],
                                         AF.Exp, scale=SCALE, bias=nbias[:])
                    next(feed, None)
                prev = (e8, fuse, j)
        with nc.named_scope("tail"):
            pe8, pfuse, pj = prev
            for nt in range(NT):
                emit_fusion(pe8, nt, MT2 - 1, pfuse)
                emit_post(pj, nt, pfuse)
    nc.compile()
    return nc


_NC = None


def _get_nc():
    global _NC
    if _NC is None:
        _NC = build()
    return _NC


def kernel(**inputs):
    x1 = np.asarray(inputs["x1"], np.float32).reshape(B, C, N)
    x2 = np.asarray(inputs["x2"], np.float32).reshape(B, C, N)
    q_w = np.asarray(inputs["q_w"], np.float32)
    k_w = np.asarray(inputs["k_w"], np.float32)
    v_w = np.asarray(inputs["v_w"], np.float32)
    p_w = np.asarray(inputs["proj_w"], np.float32)
    q_b = np.asarray(inputs["q_b"], np.float32)
    v_b = np.asarray(inputs["v_b"], np.float32)
    p_b = np.asarray(inputs["proj_b"], np.float32)
    gamma = np.asarray(inputs["bn_gamma"], np.float32)
    beta = np.asarray(inputs["bn_beta"], np.float32)
    mean = np.asarray(inputs["bn_mean"], np.float32)
    var = np.asarray(inputs["bn_var"], np.float32)
    gate_w = np.asarray(inputs["gate_w"], np.float32)
    gate_b = np.asarray(inputs["gate_b"], np.float32)

    wqk = q_w.T @ k_w                      # [C,C]
    A = gamma[:, None] / np.sqrt(var + EPS)[:, None] * (p_w @ v_w)  # G*(pw vw)
    G = gamma / np.sqrt(var + EPS)
    Bc = (beta + (p_b + p_w @ v_b - mean) * G).astype(np.float32)
    qpb = (k_w.T @ q_b).astype(np.float32)
    bct = np.ascontiguousarray(np.broadcast_to(Bc, (128, C)))

    in_maps = []
    for b in range(B):
        Qp = (wqk.T @ x1[b] + qpb[:, None]).astype(NP8)      # [C, N]
        Y8 = (A @ x2[b]).astype(NP8)                          # [C, N]
        glog = gate_w[0, :C] @ x1[b] + gate_w[0, C:] @ x2[b] + gate_b[0]
        gate = (1.0 / (1.0 + np.exp(-glog))).astype(np.float32)  # [N]
        x28 = x2[b].astype(NP8)
        # x2dr/q8 layouts: [p, h, m] = arr[h*128+p, m]
        x2dr = np.ascontiguousarray(
            x28.reshape(2, 128, N).transpose(1, 0, 2).reshape(128, 2 * N))
        yt = np.zeros((128, MT, YW), NP8)
        yt[:, :, :C] = np.ascontiguousarray(
            Y8.reshape(C, MT, 128).transpose(2, 1, 0))
        yt[:, :, C] = np.float32(1.0)
        yt = np.ascontiguousarray(yt.reshape(128, MT * YW))
        for half in range(2):
            hq = slice(half * NH, (half + 1) * NH)
            q8 = np.ascontiguousarray(
                Qp[:, hq].reshape(2, 128, NH).transpose(1, 0, 2)
                .reshape(128, 2 * NH))
            x1t = np.ascontiguousarray(
                x1[b][:, hq].T.astype(ml_dtypes.bfloat16))    # [NH, C]
            gc = np.ascontiguousarray(
                gate[hq].reshape(NBLOCKS * NT, 128).T.astype(np.float32))
            in_maps.append({
                "q8": q8, "x2dr": x2dr, "yt": yt, "x1t": x1t,
                "gatec": gc, "bct": bct,
            })

    nc = _get_nc()
    res = run_bass_kernel_spmd(nc, in_maps, core_ids=list(range(NCORES)))
    out = np.empty((B, C, N), np.float32)
    for core in range(NCORES):
        b, half = divmod(core, 2)
        out[b, :, half * NH:(half + 1) * NH] = \
            res.results[core]["out"].astype(np.float32).T
    return out.reshape(B, C, H, W)


# revision 20
# speedup vs baseline: 2.1065x; 1.0917x over previous
"""CrossAttentionFusion Trainium2 kernel — fp8 DoubleRow edition.

Full inputs -> shard (batch x query-half) over 8 NeuronCores -> full output.

Per core (batch b = core//2, query half = core%2): NH=2048 queries n,
N=4096 keys m, C=256 channels.

Host precompute (exact f32, then fp8e4 quantization):
  Q'[c,n] = (q_w^T k_w)^T x1 + k_w^T q_b        (logits rhs)
  Y[o,m]  = G[o] * (p_w v_w x2)[o,m]            (fusion rhs; G = BN scale)
  gate[n] = sigmoid(gate_w [x1;x2] + gate_b)    (per-query scalar)
  Bc[o]   = beta + (p_b + p_w v_b - mean) * G   (post bias row)
Device per 512-query block j:
  L[m,n]  = x2^T Q'      fp8 DoubleRow matmuls (contraction c=256/instr)
  E       = exp(L/16 - 2.5) on ACT, fp8e4 out (offset cancels in Z/S)
  P[n,o]  = sum_m E[m,n] Y[o,m]  fp8 DR, out in [query, channel] layout;
            Y carries a ones column so P[:,256] = S = softmax denominator
  out^T   = x1^T + gate * relu(P[:, :256]/S + Bc)   (DVE recip/STT,
            GpSimd relu*gate, DVE residual add; all per-partition scalars)
  fusion(j-1) instrs interleave into logits(j) slots on the PE; exp is the
  pacing engine (~55us floor: 8.4M elements at 128 lanes / 1.2 GHz).
"""
from contextlib import ExitStack

import numpy as np
import ml_dtypes

import concourse.bass as bass
import concourse.mybir as mybir
import concourse.tile as tile
from concourse import bacc
from concourse.bass_utils import run_bass_kernel_spmd

F32 = mybir.dt.float32
BF16 = mybir.dt.bfloat16
FP8 = mybir.dt.float8e4
AF = mybir.ActivationFunctionType
OP = mybir.AluOpType
DR = mybir.MatmulPerfMode.DoubleRow
NP8 = ml_dtypes.float8_e4m3

B, C, H, W = 4, 256, 64, 64
N = H * W            # 4096 keys per batch
NCORES = 8
NH = N // 2          # 2048 queries per core
NBLK = 512           # query block
NBLOCKS = NH // NBLK
MT = N // 128        # 32 key tiles
MT2 = MT // 2        # 16 DoubleRow key-pair steps
YW = 272             # yt row: 256 channels + ones col + pad to %16
NT = NBLK // 128     # 4 query tiles per block
EPS = 1e-5
SCALE = float(C) ** -0.5
EOFF = 2.5           # exp offset; cancels in Z/S


def build():
    nc = bacc.Bacc("TRN2", target_bir_lowering=False, debug=False,
                   num_devices=NCORES)
    q8_d = nc.dram_tensor("q8", [128, 2 * NH], FP8, kind="ExternalInput")
    x2_d = nc.dram_tensor("x2dr", [128, 2 * N], FP8, kind="ExternalInput")
    yt_d = nc.dram_tensor("yt", [128, MT * YW], FP8, kind="ExternalInput")
    x1_d = nc.dram_tensor("x1t", [NH, C], BF16, kind="ExternalInput")
    gc_d = nc.dram_tensor("gatec", [128, NBLOCKS * NT], F32,
                          kind="ExternalInput")
    bc_d = nc.dram_tensor("bct", [128, C], F32, kind="ExternalInput")
    out_d = nc.dram_tensor("out", [NH, C], BF16, kind="ExternalOutput")

    with tile.TileContext(nc) as tc, ExitStack() as ctx:
        pers = ctx.enter_context(tc.tile_pool(name="pers", bufs=1))
        epool = ctx.enter_context(tc.tile_pool(name="epool", bufs=2))
        work = ctx.enter_context(tc.tile_pool(name="work", bufs=2))
        outs = ctx.enter_context(tc.tile_pool(name="outs", bufs=3))
        psL = ctx.enter_context(tc.tile_pool(name="psL", bufs=2, space="PSUM"))
        psF = ctx.enter_context(tc.tile_pool(name="psF", bufs=4, space="PSUM"))

        q8 = pers.tile([128, 2, NH], FP8, tag="q8", name="q8")
        # chunked tiles: dependency tracking is whole-tile, so chunk tiles
        # let block-0 compute start as soon as its chunk lands
        x2c = [pers.tile([128, 2, 1024], FP8, tag=f"x2c{c}", name=f"x2c{c}")
               for c in range(4)]
        ytc = [pers.tile([128, 4, YW], FP8, tag=f"ytc{i}", name=f"ytc{i}")
               for i in range(8)]
        x1p = pers.tile([128, NBLOCKS * NT, C], BF16, tag="x1p", name="x1p")
        bct = pers.tile([128, C], F32, tag="bct", name="bct")
        gc = pers.tile([128, NBLOCKS * NT], F32, tag="gc", name="gc")
        nbias = pers.tile([128, 1], F32, tag="nbias", name="nbias")

        def _x2(c):
            c0 = c * 1024
            nc.sync.dma_start(x2c[c][:, 0, :], x2_d[:, c0:c0 + 1024])
            nc.gpsimd.dma_start(x2c[c][:, 1, :],
                                x2_d[:, N + c0:N + c0 + 1024])

        def _yt(t0):
            eng = nc.sync if (t0 // 4) % 2 == 0 else nc.gpsimd
            eng.dma_start(
                ytc[t0 // 4][:],
                yt_d[:, t0 * YW:(t0 + 4) * YW].rearrange(
                    "p (t y) -> p t y", y=YW))

        with nc.named_scope("pre"):
            nc.vector.memset(nbias[:], -EOFF)
            # PE warmup: ramp the clock out of the low p-state on scratch
            # data while the first DMAs land.
            wl = pers.tile([128, 2, 128], FP8, tag="wl", name="wl")
            wr = pers.tile([128, 2, 257], FP8, tag="wr", name="wr")
            nc.vector.memset(wl[:], 0.0)
            nc.vector.memset(wr[:], 0.0)
            for w in range(6):
                wp = psF.tile([128, 257], F32, tag="fuse", name="fuse")
                nc.tensor.matmul(wp[:], wl[:], wr[:], start=True, stop=True,
                                 perf_mode=DR)
            nc.sync.dma_start(q8[:, 0, :], q8_d[:, 0:NH])
            nc.gpsimd.dma_start(q8[:, 1, :], q8_d[:, NH:2 * NH])
            _x2(0)

        def dma_feed():
            # Deferred DMA issues, one step per slot, just ahead of need.
            # Consumers wait on all prior issues of a queue, so issuing late
            # (but before the consuming instr is emitted) is what overlaps
            # transfers with block-0 compute.
            _x2(1); _yt(0); _yt(4)
            yield
            _x2(2)
            yield
            _yt(8); _yt(12)
            yield
            _x2(3)
            yield
            _yt(16); _yt(20)
            yield
            _yt(24); _yt(28)
            yield
            nc.sync.dma_start(bct[:], bc_d[:])
            nc.sync.dma_start(gc[:], gc_d[:])
            yield
            for t in range(NBLOCKS * NT):
                eng = nc.gpsimd if t % 2 == 0 else nc.sync
                eng.dma_start(x1p[:, t, :], x1_d[t * 128:(t + 1) * 128, :])
                if t % 2 == 1:
                    yield

        feed = dma_feed()

        def emit_fusion(e8, nt, k, fuse, fcnt):
            if fcnt[nt] == 0:
                fuse[nt] = psF.tile([128, 257], F32, tag="fuse", name="fuse")
            sub = (2 * k) % 4
            nc.tensor.matmul(
                fuse[nt][:, 0:257],
                e8[:, 2 * k:2 * k + 2, nt * 128:(nt + 1) * 128],
                ytc[k // 2][:, sub:sub + 2, 0:257],
                start=(fcnt[nt] == 0), stop=(fcnt[nt] == MT2 - 1),
                perf_mode=DR)
            fcnt[nt] += 1

        def emit_posts(j, fuse):
            # copy PSUM out first (releases the fuse banks for the next
            # block's accumulators), then the per-nt postludes
            cps = []
            with nc.named_scope(f"post{j}"):
                for nt in range(NT):
                    cp = work.tile([128, 257], F32, tag=f"cp{nt}",
                                   name=f"cp{nt}")
                    nc.vector.tensor_copy(cp[:], fuse[nt][:])
                    cps.append(cp)
                for nt in range(NT):
                    t_idx = j * NT + nt
                    cp = cps[nt]
                    invs = work.tile([128, 1], F32, tag="invs", name="invs")
                    nc.vector.reciprocal_approx_fast(invs[:], cp[:, 256:257])
                    tt = work.tile([128, C], F32, tag="tt", name="tt")
                    nc.vector.scalar_tensor_tensor(
                        tt[:], cp[:, 0:256], invs[:], bct[:],
                        op0=OP.mult, op1=OP.add)
                    rg = work.tile([128, C], F32, tag="rg", name="rg")
                    nc.vector.tensor_scalar_max(rg[:], tt[:], 0.0)
                    ot = outs.tile([128, C], BF16, tag="ot", name="ot")
                    nc.vector.scalar_tensor_tensor(
                        ot[:], rg[:], gc[:, t_idx:t_idx + 1],
                        x1p[:, t_idx, :], op0=OP.mult, op1=OP.add)
                    nc.sync.dma_start(
                        out_d[t_idx * 128:(t_idx + 1) * 128, :], ot[:])

        # Hybrid exp: most slots on ACT; DVE_KS slots use the Schraudolph
        # bit-trick (y*2^23/ln2 + magic as int32, bitcast = approx exp).
        DVE_KS = (8, 11, 14)
        A_S = (8388608.0 / float(np.log(2.0))) * SCALE
        B_S = 1065353216.0 - 366393.0 - EOFF * (8388608.0 / float(np.log(2.0)))

        def emit_exp(e8, mt2, lp):
            if mt2 in DVE_KS:
                it = work.tile([128, 2, NBLK], mybir.dt.int32, tag="i32",
                               name="i32")
                nc.vector.tensor_scalar(it[:], lp[:], A_S, B_S,
                                        op0=OP.mult, op1=OP.add)
                nc.vector.tensor_copy(e8[:, 2 * mt2:2 * mt2 + 2, :],
                                      it[:].bitcast(F32))
            else:
                nc.scalar.activation(e8[:, 2 * mt2:2 * mt2 + 2, :], lp[:],
                                     AF.Exp, scale=SCALE, bias=nbias[:])

        def ks_for_slot(s):
            # fusion k emitted at slot s: lag 2 for ACT slots, 3 for DVE
            # slots (their E is ready one slot later); ACT-k first
            ks = []
            if 0 <= s - 2 < MT2 and (s - 2) not in DVE_KS:
                ks.append(s - 2)
            if 0 <= s - 3 < MT2 and (s - 3) in DVE_KS:
                ks.append(s - 3)
            return ks

        prev = None  # (e8, fuse, fcnt, j): spillover ks + posts pending
        for j in range(NBLOCKS):
            ns = slice(j * NBLK, (j + 1) * NBLK)
            with nc.named_scope(f"blk{j}"):
                e8 = epool.tile([128, MT, NBLK], FP8, tag="E8", name="E8")
                fuse = {}
                fcnt = [0] * NT
                for mt2 in range(MT2):
                    lp = psL.tile([128, 2, NBLK], F32, tag="L", name="L")
                    for sub in range(2):
                        mt = 2 * mt2 + sub
                        cc = (mt % 8) * 128
                        nc.tensor.matmul(
                            lp[:, sub, :],
                            x2c[mt // 8][:, :, cc:cc + 128],
                            q8[:, :, ns], start=True, stop=True, perf_mode=DR)
                    if prev is not None and mt2 <= 1:
                        pe8, pfuse, pfcnt, pj = prev
                        for k in ks_for_slot(MT2 + mt2):
                            for nt in range(NT):
                                emit_fusion(pe8, nt, k, pfuse, pfcnt)
                        if mt2 == 1:
                            emit_posts(pj, pfuse)
                            prev = None
                    for k in ks_for_slot(mt2):
                        for nt in range(NT):
                            emit_fusion(e8, nt, k, fuse, fcnt)
                    emit_exp(e8, mt2, lp)
                    next(feed, None)
                prev = (e8, fuse, fcnt, j)
        with nc.named_scope("tail"):
            pe8, pfuse, pfcnt, pj = prev
            for s in (MT2, MT2 + 1):
                for k in ks_for_slot(s):
                    for nt in range(NT):
                        emit_fusion(pe8, nt, k, pfuse, pfcnt)
            emit_posts(pj, pfuse)
    nc.compile()
    return nc


_NC = None


def _get_nc():
    global _NC
    if _NC is None:
        _NC = build()
    return _NC


def kernel(**inputs):
    x1 = np.asarray(inputs["x1"], np.float32).reshape(B, C, N)
    x2 = np.asarray(inputs["x2"], np.float32).reshape(B, C, N)
    q_w = np.asarray(inputs["q_w"], np.float32)
    k_w = np.asarray(inputs["k_w"], np.float32)
    v_w = np.asarray(inputs["v_w"], np.float32)
    p_w = np.asarray(inputs["proj_w"], np.float32)
    q_b = np.asarray(inputs["q_b"], np.float32)
    v_b = np.asarray(inputs["v_b"], np.float32)
    p_b = np.asarray(inputs["proj_b"], np.float32)
    gamma = np.asarray(inputs["bn_gamma"], np.float32)
    beta = np.asarray(inputs["bn_beta"], np.float32)
    mean = np.asarray(inputs["bn_mean"], np.float32)
    var = np.asarray(inputs["bn_var"], np.float32)
    gate_w = np.asarray(inputs["gate_w"], np.float32)
    gate_b = np.asarray(inputs["gate_b"], np.float32)

    wqk = q_w.T @ k_w                      # [C,C]
    A = gamma[:, None] / np.sqrt(var + EPS)[:, None] * (p_w @ v_w)  # G*(pw vw)
    G = gamma / np.sqrt(var + EPS)
    Bc = (beta + (p_b + p_w @ v_b - mean) * G).astype(np.float32)
    qpb = (k_w.T @ q_b).astype(np.float32)
    bct = np.ascontiguousarray(np.broadcast_to(Bc, (128, C)))

    in_maps = []
    for b in range(B):
        Qp = (wqk.T @ x1[b] + qpb[:, None]).astype(NP8)      # [C, N]
        Y8 = (A @ x2[b]).astype(NP8)                          # [C, N]
        glog = gate_w[0, :C] @ x1[b] + gate_w[0, C:] @ x2[b] + gate_b[0]
        gate = (1.0 / (1.0 + np.exp(-glog))).astype(np.float32)  # [N]
        x28 = x2[b].astype(NP8)
        # x2dr/q8 layouts: [p, h, m] = arr[h*128+p, m]
        x2dr = np.ascontiguousarray(
            x28.reshape(2, 128, N).transpose(1, 0, 2).reshape(128, 2 * N))
        yt = np.zeros((128, MT, YW), NP8)
        yt[:, :, :C] = np.ascontiguousarray(
            Y8.reshape(C, MT, 128).transpose(2, 1, 0))
        yt[:, :, C] = np.float32(1.0)
        yt = np.ascontiguousarray(yt.reshape(128, MT * YW))
        for half in range(2):
            hq = slice(half * NH, (half + 1) * NH)
            q8 = np.ascontiguousarray(
                Qp[:, hq].reshape(2, 128, NH).transpose(1, 0, 2)
                .reshape(128, 2 * NH))
            x1t = np.ascontiguousarray(
                x1[b][:, hq].T.astype(ml_dtypes.bfloat16))    # [NH, C]
            gc = np.ascontiguousarray(
                gate[hq].reshape(NBLOCKS * NT, 128).T.astype(np.float32))
            in_maps.append({
                "q8": q8, "x2dr": x2dr, "yt": yt, "x1t": x1t,
                "gatec": gc, "bct": bct,
            })

    nc = _get_nc()
    res = run_bass_kernel_spmd(nc, in_maps, core_ids=list(range(NCORES)))
    out = np.empty((B, C, N), np.float32)
    for core in range(NCORES):
        b, half = divmod(core, 2)
        out[b, :, half * NH:(half + 1) * NH] = \
            res.results[core]["out"].astype(np.float32).T
    return out.reshape(B, C, H, W)
